# revision 24
# baseline (speedup 1.0000x reference)
"""Multi-head self-attention (B=4, S=2048, D=1024, H=16) on 8 trn2 NeuronCores.

Sharding: batch (4) x head-group (2 groups of 8 heads) -> 8 cores.
Each core computes, for its (batch b, head-group hg):
  Q'^T = (wq_l/8) @ x_b^T            [512, 2048]   (1/sqrt(dk) folded into wq)
  K^T  = wk_l @ x_b^T                [512, 2048]
  V    = x_b @ wv_l^T                [2048, 512]
  per head h (8 local, dk=64), in transposed layout (keys on partitions):
    scoresT[k, q] = K_h @ Q'_h^T     (no max-subtraction: scores ~ N(0,4))
    expT = exp(scoresT)              (ScalarE, PSUM->SBUF bf16)
    unnormT[c, q] = V_h^T @ expT     (PE, accumulated over key tiles)
    Z[q] = ones^T @ expT             (PE colsum quads, same accumulation)
    attnT = unnormT / Z              (DVE mul straight out of PSUM)
  out_partial = attnT^T @ wo_l^T     [2048, 1024]  (row-parallel wo)
Host upcasts the bf16 partials and sums the two per batch.

Schedule: the 256 exps (ScalarE, ~1.11us each, 284.6us total) are the
floor.  Per key-tile the PE needs ~1.9us (scores row-paired by head, V
col-paired, colsum quads) vs 2.29us of ACT; the ~0.4us/kt of slack
absorbs, via a static tick schedule, the Q/K projections for g>=1 and the
qh1 s-chunks, plus the qh0 half of the output projection.  Blocks run
qh-outer (qh0: g0..g3, then qh1: g0..g3) so that wo half overlaps the
second half of attention; only the qh1 wo chunks trail the last block.
Attention starts ~29us in, as soon as K/Q g0 and V are projected, which
itself starts after 1 (of 4) x quarter-loads.

attn is stored [128, j, S] per g2=g//2 with channel c = g*128 + hp*64 + r
at partition hp*64+r, slot j=g%2: both normalization halves write their
natural partitions (hp1's V-matmul output lands on partitions 64-127) and
the wo matmul contracts [128]-partition slices per (g2, j) -- no
partition-crossing moves anywhere.

Normalization is off-path and cheap: reciprocal_approx_fast straight from
the colsum PSUM bank, denominators broadcast across partitions via a
DRAM-roundtrip DMA, numerator read directly from the V-accumulation PSUM.
"""

import ml_dtypes
import numpy as np

import bass_rust
import concourse.bass as bass
import concourse.mybir as mybir
import concourse.tile as tile

# ---------------------------------------------------------------- constants
S = 2048          # sequence length
DM = 1024         # model dim
DL = 512          # local (per-core) head dims = 8 heads * 64
DK = 64           # head dim
P = 128
NKT = S // P      # 16 key tiles
NG = DL // P      # 4 head-pairs (c-tiles / dq-tiles)
KD = DM // P      # 8 contraction tiles for projections
NSC = S // 512    # 4 s-chunks for projections
F32 = mybir.dt.float32
BF16 = mybir.dt.bfloat16
BF16_NP = ml_dtypes.bfloat16

N_CORES = 8
CORE_IDS = list(range(N_CORES))


# ------------------------------------------------- walrus sync-wait workaround
def _split_sync_waits(nc, limit=1):
    """This toolchain's walrus codegen rejects instructions carrying more than
    one sync-wait command.  Move excess waits onto dedicated same-engine nops
    inserted immediately before the instruction (sequential waits on the same
    engine queue are semantically identical to multiple waits on one inst)."""
    fn = nc.m.functions[0]
    snapshots = [(bb, list(bb.instructions)) for bb in fn.blocks]
    plans = []
    for _bb, insts in snapshots:
        plan = {}
        for idx, inst in enumerate(insts):
            si = inst.sync_info
            waits = list(si.on_wait) if si and si.on_wait else []
            if len(waits) > limit:
                pre, keep = waits[:-limit], waits[-limit:]
                nops = []
                for w in pre:
                    ni = nc.engines[inst.engine].nop(nofuse=True, hint="wsplit").ins
                    ni.sync_info = bass_rust.SyncInfo(on_wait=[w], on_update=[])
                    nops.append(ni)
                si.on_wait = keep
                plan[idx] = nops
        plans.append(plan)
    for (bb, insts), plan in zip(snapshots, plans):
        out = []
        for idx, inst in enumerate(insts):
            out.extend(plan.get(idx, ()))
            out.append(inst)
        bb.instructions = out


# ---------------------------------------------------------------- the program
def build_nc():
    """Build the SPMD per-core Bass program (identical on all 8 cores)."""
    nc = bass.Bass()

    xT = nc.declare_dram_parameter("xT", [DM, S], BF16, isOutput=False)
    wqT = nc.declare_dram_parameter("wqT", [DM, DL], BF16, isOutput=False)
    wkT = nc.declare_dram_parameter("wkT", [DM, DL], BF16, isOutput=False)
    wvT = nc.declare_dram_parameter("wvT", [DM, DL], BF16, isOutput=False)
    # wo packed [128, g2, j, DM]: row p, slot (g2,j) holds channel
    # c = g2*256 + j*128 + p of wo^T (matches the attn storage layout)
    woP = nc.declare_dram_parameter("woP", [P, 2, 2, DM], BF16, isOutput=False)
    out = nc.declare_dram_parameter("out", [S, DM], BF16, isOutput=True)

    with tile.TileContext(nc) as tc:
        with (
            tc.tile_pool(name="big", bufs=1) as big,
            tc.tile_pool(name="expT", bufs=8) as expp,
            tc.tile_pool(name="rc", bufs=2) as rcp,
            tc.tile_pool(name="outsb", bufs=3) as outp,
            tc.tile_pool(name="dram", bufs=2, space="DRAM") as dramp,
            tc.tile_pool(name="ps", bufs=2, space="PSUM") as psp,
            tc.tile_pool(name="av", bufs=3, space="PSUM") as avp,
            tc.tile_pool(name="cs", bufs=1, space="PSUM") as csp,
        ):
            # ---------------- constants + ACT exp-table preload (~2.7us,
            # paid during the DMA wait instead of on the first real exp)
            ones_bf = big.tile([P, 1], BF16, tag="ones")
            nc.vector.memset(ones_bf[:], 1.0)
            warm = big.tile([1, 1], BF16, tag="warm")
            nc.scalar.activation(
                warm[:], ones_bf[0:1, 0:1], mybir.ActivationFunctionType.Exp
            )

            # ---------------- DMA loads, ordered by first use
            w_sb = {}
            w_sb["wk"] = big.tile([P, KD, DL], BF16, tag="wk", name="wk")
            nc.sync.dma_start(
                w_sb["wk"][:], wkT.rearrange("(kd p) m -> p kd m", p=P)
            )
            xT_r = xT.rearrange("(kd p) s -> p kd s", p=P)
            xT_q = [
                big.tile([P, KD, 512], BF16, tag=f"xT{j}", name=f"xTq{j}")
                for j in range(4)
            ]
            nc.sync.dma_start(xT_q[0][:], xT_r[:, :, 0:512])
            w_sb["wq"] = big.tile([P, KD, DL], BF16, tag="wq", name="wq")
            nc.sync.dma_start(
                w_sb["wq"][:], wqT.rearrange("(kd p) m -> p kd m", p=P)
            )
            w_sb["wv"] = big.tile([P, KD, DL], BF16, tag="wv", name="wv")
            nc.sync.dma_start(
                w_sb["wv"][:], wvT.rearrange("(kd p) m -> p kd m", p=P)
            )
            nc.sync.dma_start(xT_q[1][:], xT_r[:, :, 512:1024])
            nc.sync.dma_start(xT_q[2][:], xT_r[:, :, 1024:1536])
            nc.sync.dma_start(xT_q[3][:], xT_r[:, :, 1536:2048])
            woP_sb = big.tile([P, 2, 2, DM], BF16, tag="woP")
            nc.sync.dma_start(woP_sb[:], woP[:])

            def xslice(kd, fr, to):
                q = fr // 512
                assert to <= (q + 1) * 512
                return xT_q[q][:, kd, fr - q * 512 : to - q * 512]

            # persistent activation tensors
            QT = [big.tile([P, S], BF16, tag=f"QT{g}", name=f"QT{g}") for g in range(NG)]
            KT = [big.tile([P, S], BF16, tag=f"KT{g}", name=f"KT{g}") for g in range(NG)]
            V_st = [big.tile([P, 8, DK + 1], BF16, tag=f"V{st}", name=f"V{st}") for st in range(NKT)]
            # attnP[g2]: [128, j, S]; channel c = g*128+hp*64+r lives at
            # [hp*64+r, g%2] of tile g//2 (partition-aligned with vt PSUM)
            attnP = [
                big.tile([P, 2, S], BF16, tag=f"attn{g2}", name=f"attn{g2}")
                for g2 in range(2)
            ]

            # ---------------- chunk emitters
            def proj_qk_chunk(dst, w, g, sc, pool, tag):
                ps = pool.tile([P, 512], F32, tag=tag, name="projch")
                for kd in range(KD):
                    nc.tensor.matmul(
                        ps[:],
                        lhsT=w[:, kd, g * P : (g + 1) * P],
                        rhs=xslice(kd, sc * 512, (sc + 1) * 512),
                        start=(kd == 0),
                        stop=(kd == KD - 1),
                    )
                nc.vector.tensor_copy(out=dst[:, sc * 512 : (sc + 1) * 512], in_=ps[:])

            def proj_v(st, pool, tag):
                ps = pool.tile([P, 512], F32, tag=tag, name="projv")
                for kd in range(KD):
                    nc.tensor.matmul(
                        ps[:],
                        lhsT=xslice(kd, st * P, (st + 1) * P),
                        rhs=w_sb["wv"][:, kd, :],
                        start=(kd == 0),
                        stop=(kd == KD - 1),
                    )
                nc.vector.tensor_copy(
                    out=V_st[st][:, :, 0:DK],
                    in_=ps.rearrange("p (h c) -> p h c", c=DK),
                )

            def wo_chunk(st, ob, pool, tag):
                ps = pool.tile([P, 512], F32, tag=tag, name="wochunk")
                k = 0
                for g2 in range(2):
                    for j in range(2):
                        nc.tensor.matmul(
                            ps[:],
                            lhsT=attnP[g2][:, j, st * P : (st + 1) * P],
                            rhs=woP_sb[:, g2, j, ob * 512 : (ob + 1) * 512],
                            start=(k == 0),
                            stop=(k == 3),
                        )
                        k += 1
                ot = outp.tile([P, 512], BF16, tag="out")
                nc.vector.tensor_copy(out=ot[:], in_=ps[:])
                nc.sync.dma_start(
                    out[st * P : (st + 1) * P, ob * 512 : (ob + 1) * 512], ot[:]
                )

            # ---------------- prologue: K/Q g0 (all the first 2 s-chunks
            # need) + the full V projection, gated by x-quarter arrival
            proj_qk_chunk(KT[0], w_sb["wk"], 0, 0, psp, "ps")
            proj_qk_chunk(QT[0], w_sb["wq"], 0, 0, psp, "ps")
            for st in range(4):
                proj_v(st, psp, "ps")
            proj_qk_chunk(KT[0], w_sb["wk"], 0, 1, psp, "ps")
            proj_qk_chunk(QT[0], w_sb["wq"], 0, 1, psp, "ps")
            for st in range(4, 8):
                proj_v(st, psp, "ps")

            # ---------------- attention
            class AttnBlock:
                """Heads A=2g, B=2g+1; query half qh (1024 queries).

                scoresT/exp are ACT-paced.  V and colsum matmuls lag one kt
                behind (carried across block boundaries by the driver loop) so
                both heads' exp tiles are ready together, letting adjacently
                issued matmuls with disjoint array tile positions (V: col
                groups 0-1 vs 2-3; colsums: 32-strips 0/32/64/96) run
                concurrently on the PE.  vt accumulates A in partitions 0-63
                and B in 64-127 of one bank (memset + start=False keeps the
                interleaved accumulation groups from clearing each other).
                Normalization runs off the critical path, reading vt/cs PSUM
                directly; the attnP layout keeps it partition-aligned.
                """

                def __init__(self, g, qh):
                    self.g, self.qoff = g, qh * 1024
                    self.vt = None
                    self.ets = {}

                def ensure_vt(self):
                    # Allocated lazily at the first V matmul (one tick after
                    # the block starts) so the slot-WAR memsets queue after
                    # the previous block's PSUM drains, not before.
                    if self.vt is None:
                        self.vt = [
                            avp.tile([P, 512], F32, tag="av", name=f"vt{qb}")
                            for qb in range(2)
                        ]
                        self.cs = csp.tile([P, 512], F32, tag="cs")
                        for t in self.vt:
                            nc.vector.memset(t[:], 0.0)
                        nc.vector.memset(self.cs[:], 0.0)

                def emit_scores_exp(self, kt):
                    g, qoff = self.g, self.qoff
                    for hp, pb in ((0, 0), (1, 64)):
                        ps_s = psp.tile([P, 1024], F32, tag="ps", name=f"ps_s{hp}")
                        for qb in range(2):
                            nc.tensor.matmul(
                                ps_s[:, qb * 512 : (qb + 1) * 512],
                                lhsT=KT[g][pb : pb + 64, kt * P : (kt + 1) * P],
                                rhs=QT[g][
                                    pb : pb + 64,
                                    qoff + qb * 512 : qoff + (qb + 1) * 512,
                                ],
                                start=True,
                                stop=True,
                            )
                        et = expp.tile([P, 1024], BF16, tag="expT", name=f"et{hp}")
                        nc.scalar.activation(
                            et[:], ps_s[:], mybir.ActivationFunctionType.Exp
                        )
                        self.ets[(kt, hp)] = et

                def emit_v_cs(self, kt):
                    g = self.g
                    last = kt == NKT - 1
                    self.ensure_vt()
                    et = {hp: self.ets.pop((kt, hp)) for hp in (0, 1)}
                    for qb in range(2):
                        for hp, pb in ((0, 0), (1, 64)):
                            nc.tensor.matmul(
                                self.vt[qb][pb : pb + 64, :],
                                lhsT=V_st[kt][:, 2 * g + hp, 0:DK],
                                rhs=et[hp][:, qb * 512 : (qb + 1) * 512],
                                start=False,
                                stop=last,
                                skip_group_check=True,
                                tile_position=(0, pb),
                            )
                    for hp in (0, 1):
                        for qb in range(2):
                            cp = 64 * hp + 32 * qb
                            nc.tensor.matmul(
                                self.cs[cp : cp + 1, :],
                                lhsT=ones_bf[:],
                                rhs=et[hp][:, qb * 512 : (qb + 1) * 512],
                                start=False,
                                stop=last,
                                skip_group_check=True,
                                tile_position=(0, cp),
                            )
                    if last:
                        return self.emit_norm_a()
                    return None

                def emit_norm_a(self):
                    """Drain vt/cs PSUM to SBUF (DVE-only chain, so the next
                    block's PSUM slot reuse never waits on a DMA) and launch
                    the zd round trip.  Returns a closure for the
                    DMA-dependent half, to be emitted a couple of ticks later
                    so the waiting muls don't plug the in-order DVE queue."""
                    g, qoff = self.g, self.qoff
                    g2, j = g // 2, g % 2
                    un = [
                        rcp.tile([P, 512], F32, tag=f"un{qb}", name=f"un{qb}")
                        for qb in range(2)
                    ]
                    for qb in range(2):
                        nc.vector.tensor_copy(out=un[qb][:], in_=self.vt[qb][:])
                    # 1/colsum straight from PSUM (junk rows harmless; only
                    # rows {0,32,64,96} are read back)
                    cs_rc = rcp.tile([P, 512], F32, tag="cs_rc")
                    nc.vector.reciprocal(cs_rc[:], self.cs[:])
                    zd = dramp.tile([4, 512], F32, name="zd")
                    # zd rows: 0=(A,qb0) 1=(A,qb1) 2=(B,qb0) 3=(B,qb1)
                    nc.sync.dma_start(zd[:], cs_rc[0:128:32, :])

                    def norm_b():
                        for qb in range(2):
                            rcb = rcp.tile(
                                [P, 512], F32, tag=f"rcb{qb}", name=f"rcb{qb}"
                            )
                            nc.sync.dma_start(
                                rcb[0:64, :],
                                zd[qb, None, :].to_broadcast([64, 512]),
                            )
                            nc.sync.dma_start(
                                rcb[64:128, :],
                                zd[qb + 2, None, :].to_broadcast([64, 512]),
                            )
                            nc.vector.tensor_mul(
                                out=attnP[g2][
                                    :, j, qoff + qb * 512 : qoff + (qb + 1) * 512
                                ],
                                in0=un[qb][:],
                                in1=rcb[:],
                            )

                    return norm_b

            # ---------------- static injection schedule (tick -> emitters).
            # Hard deadlines (PE queue is in-order): K[g0] s2 by tick 7
            # (kt8 scores), s3 by 11; K/Q[g] (qh0 cols) by tick 16g; Q[g]
            # s2-3 by tick 64+16g; wo(st<8) after the qh0 norms (tick >=66).
            K, Q = "K", "Q"
            sched = {}

            def add(tick, *item):
                sched.setdefault(tick, []).append(item)

            # early ticks carry V st8-15 (hard deadline: st consumed at tick
            # st+1) interleaved with the K/Q chunks blocks g0/g1 need
            add(1, "V", 8)
            add(2, "V", 9)
            add(3, "V", 10)
            add(4, "V", 11)
            add(5, K, 0, 2)
            add(6, K, 1, 0)
            add(7, "V", 12)
            add(8, K, 1, 1)
            add(9, K, 0, 3)
            add(10, "V", 13)
            add(11, K, 1, 2)
            add(12, "V", 14)
            add(13, K, 1, 3)
            add(14, Q, 1, 0)
            add(14, Q, 1, 1)
            add(15, "V", 15)
            for i, t in enumerate((17, 19, 21, 23)):
                add(t, K, 2, i)
            add(26, Q, 2, 0)
            add(28, Q, 2, 1)
            for i, t in enumerate((33, 35, 37, 39)):
                add(t, K, 3, i)
            add(42, Q, 3, 0)
            add(44, Q, 3, 1)
            t = 47
            for g in range(NG):
                for sc in (2, 3):
                    add(t, Q, g, sc)
                    t += 2
            # wo chunks for qh0 (st 0-7), injected during qh1
            t = 68
            for st in range(8):
                for ob in range(2):
                    add(t, "WO", st, ob)
                    t += 2

            def run_items(items):
                for item in items:
                    kind = item[0]
                    if kind == K:
                        proj_qk_chunk(KT[item[1]], w_sb["wk"], item[1], item[2], avp, "av")
                    elif kind == Q:
                        proj_qk_chunk(QT[item[1]], w_sb["wq"], item[1], item[2], avp, "av")
                    elif kind == "V":
                        proj_v(item[1], avp, "av")
                    else:
                        wo_chunk(item[1], item[2], avp, "av")

            # ---------------- driver: qh-outer so the qh0 half of the output
            # projection overlaps the qh1 half of attention
            pending = None
            gkt = 0
            deferred = []  # (due_tick, closure) for norm_b halves
            for qh in range(2):
                for g in range(NG):
                    blk = AttnBlock(g, qh)
                    for kt in range(NKT):
                        blk.emit_scores_exp(kt)
                        while deferred and deferred[0][0] <= gkt:
                            deferred.pop(0)[1]()
                        if pending is not None:
                            nb = pending[0].emit_v_cs(pending[1])
                            if nb is not None:
                                deferred.append((gkt + 2, nb))
                        pending = (blk, kt)
                        run_items(sched.pop(gkt, ()))
                        gkt += 1
            nb = pending[0].emit_v_cs(pending[1])
            for _, fn in deferred:
                fn()
            if nb is not None:
                nb()
            for tick in sorted(sched):
                run_items(sched[tick])

            # ---------------- output projection for qh1 (the tail):
            # chunk pairs with interleaved matmuls into the two ps slots so
            # the PE pipelines two independent accumulation chains
            def wo_pair(c1, c2):
                pss = [
                    psp.tile([P, 512], F32, tag="ps", name=f"wo{i}")
                    for i in range(2)
                ]
                for g2 in range(2):
                    for j in range(2):
                        for ps, (st, ob) in zip(pss, (c1, c2)):
                            nc.tensor.matmul(
                                ps[:],
                                lhsT=attnP[g2][:, j, st * P : (st + 1) * P],
                                rhs=woP_sb[:, g2, j, ob * 512 : (ob + 1) * 512],
                                start=(g2 == 0 and j == 0),
                                stop=(g2 == 1 and j == 1),
                                skip_group_check=True,
                            )
                for ps, (st, ob) in zip(pss, (c1, c2)):
                    ot = outp.tile([P, 512], BF16, tag="out")
                    nc.vector.tensor_copy(out=ot[:], in_=ps[:])
                    nc.sync.dma_start(
                        out[st * P : (st + 1) * P, ob * 512 : (ob + 1) * 512],
                        ot[:],
                    )

            for st in range(8, NKT):
                wo_pair((st, 0), (st, 1))

    _split_sync_waits(nc)
    return nc


_NC = None


def _get_nc():
    global _NC
    if _NC is None:
        _NC = build_nc()
    return _NC


# ---------------------------------------------------------------- host side
def make_in_maps(x, wq, wk, wv, wo):
    x = np.asarray(x, dtype=np.float32)
    wq = np.asarray(wq, dtype=np.float32)
    wk = np.asarray(wk, dtype=np.float32)
    wv = np.asarray(wv, dtype=np.float32)
    wo = np.asarray(wo, dtype=np.float32)
    in_maps = []
    for c in range(N_CORES):
        b, hg = c // 2, c % 2
        sl = slice(hg * DL, (hg + 1) * DL)
        xTc = np.ascontiguousarray(x[b].T).astype(BF16_NP)
        wqTc = np.ascontiguousarray((wq[sl] / 8.0).T).astype(BF16_NP)
        wkTc = np.ascontiguousarray(wk[sl].T).astype(BF16_NP)
        wvTc = np.ascontiguousarray(wv[sl].T).astype(BF16_NP)
        # woP: [128 p, g2, j, DM]; channel c = g2*256 + j*128 + p
        woT = np.ascontiguousarray(wo[:, sl].T)  # [DL, DM]
        woPc = np.ascontiguousarray(
            woT.reshape(2, 2, P, DM).transpose(2, 0, 1, 3)
        ).astype(BF16_NP)
        in_maps.append(
            {"xT": xTc, "wqT": wqTc, "wkT": wkTc, "wvT": wvTc, "woP": woPc}
        )
    return in_maps


def gather(results):
    out = np.zeros((4, S, DM), dtype=np.float32)
    for c in range(N_CORES):
        out[c // 2] += results[c]["out"].astype(np.float32)
    return out


def kernel(x, wq, wk, wv, wo):
    from concourse.bass_utils import run_bass_kernel_spmd

    nc = _get_nc()
    in_maps = make_in_maps(x, wq, wk, wv, wo)
    res = run_bass_kernel_spmd(nc, in_maps, CORE_IDS)
    return gather(res.results)
